# revision 2
# baseline (speedup 1.0000x reference)
"""Self-contained Trainium2 Bass kernel for the 2-layer GAT problem.

Accepts FULL inputs (as produced by setup_inputs()), shards across the
8 NeuronCores internally (dst-sharded edges), returns the full
[100000, 1] float32 output.

Design: host index preprocessing builds, per core, per-partition edge
streams (partition = (src-group, dst-range), dst-sorted) plus static
local_scatter index maps. On device, node->edge-stream expansion is
done with gpsimd local_scatter (per-partition indices) + DVE
segmented-broadcast scans; per-dst softmax sums are extracted by
scattering scan values at run-ends and reducing over groups with an
indicator matmul on the PE. Self-loops are handled analytically as
node-wise terms. f32 values cross the 2-byte scatter path as bf16
hi/lo pairs where precision matters. All weight-derived scalars are
baked into the program at build time.

Falls back to a numpy reference path if b1 != 0 or N % 8 != 0.
"""
import numpy as np
import ml_dtypes

# ===================== tile drain workaround =====================
"""This walrus build allows at most 1 sync-wait on SP CTRL instructions,
but TileContext's tail drain aggregates all end-of-kernel waits onto one
Drain. Split them across nops instead."""
import concourse.tile as tile
from concourse import mybir
from bass_rust import ScopedClock


def _patched_drain_and_barrier(self, tick_clock, wait_clock):
    nc = self.nc
    probe = nc.sync.nop()
    wait_clock.add_sem_waits(probe.ins, ScopedClock({None: tick_clock.global_clock}))
    si = probe.ins.sync_info
    waits = list(si.on_wait) if si is not None else []
    if si is not None:
        si.on_wait = waits[:1]
    for w in waits[1:]:
        nop = nc.sync.nop()
        nop.ins.sync_info = mybir.SyncInfo(on_wait=[w], on_update=[])
    nc.sync.drain()
    nc.all_engine_barrier()
    popped = nc._tile_sem_poison_stack.pop()
    assert popped is self._sem_poison
    nc.clear_and_free_semaphores(list(self.sems.allocated().values()))
    nc.all_engine_barrier()


def install():
    tile.TileContext._drain_and_barrier = _patched_drain_and_barrier


# ===================== host preprocessing =====================
BF = ml_dtypes.bfloat16
NC = 8      # cores (dst shards)
NG = 8      # src groups
NSC = 16    # dst ranges per core
DR = 784    # dsts per range (12544 padded shard)
DRH = 392
GH = 6272   # src-group half size (padded)
NSHP = NSC * DR


def _ceil16(v):
    return int(-(-int(v) // 16) * 16)


def prep2(edge_index, N):
    Nsh = N // NC
    src = np.asarray(edge_index[0], np.int64)
    dst = np.asarray(edge_index[1], np.int64)
    E = src.shape[0]
    core = dst // Nsh
    g = src // Nsh
    dl = (dst - core * Nsh).astype(np.int64)
    sl = (src - g * Nsh).astype(np.int64)
    part = g * NSC + dl // DR
    mh = (dl % DR) // DRH
    ah = sl // GH
    cellm = (core * 128 + part) * 2 + mh
    cella = (core * 128 + part) * 2 + ah

    # main (dst-sorted) streams
    om = np.lexsort((sl, dl, cellm))
    cm, dm = cellm[om], dl[om]
    counts_m = np.bincount(cm, minlength=NC * 256)
    Lh = _ceil16(counts_m.max() + 1)
    assert Lh <= 2046, f"Lh={Lh} exceeds local_scatter limit"
    starts_m = np.zeros(NC * 256, np.int64)
    np.cumsum(counts_m[:-1], out=starts_m[1:])
    pos_m = np.arange(E, dtype=np.int64) - starts_m[cm]
    same_m = np.zeros(E, bool)
    same_m[1:] = (cm[1:] == cm[:-1]) & (dm[1:] == dm[:-1])
    first_m = ~same_m
    last_m = np.ones(E, bool)
    last_m[:-1] = ~same_m[1:]

    rowm = (cm // 2) % 128
    corem = cm // 256
    colm = (cm % 2) * Lh + pos_m

    main_a = np.zeros((NC, 128, 2 * Lh), np.float32)
    main_a[corem[same_m], rowm[same_m], colm[same_m]] = 1.0
    ends_idx = np.full((NC, 128, 2 * Lh), -1, np.int16)
    ends_idx[corem[last_m], rowm[last_m], colm[last_m]] = (
        dm[last_m] % DR).astype(np.int16)
    vplace = np.full((NC, 2, 128, DRH), -1, np.int16)
    fm = first_m
    vplace[corem[fm], (cm % 2)[fm], rowm[fm],
           ((dm % DR) % DRH)[fm]] = pos_m[fm].astype(np.int16)

    # aux (src-sorted) streams
    oa = np.lexsort((dl, sl, cella))
    ca, sa = cella[oa], sl[oa]
    counts_a = np.bincount(ca, minlength=NC * 256)
    La = _ceil16(counts_a.max() + 1)
    assert La <= 2046, f"La={La}"
    starts_a = np.zeros(NC * 256, np.int64)
    np.cumsum(counts_a[:-1], out=starts_a[1:])
    pos_a = np.arange(E, dtype=np.int64) - starts_a[ca]
    same_a = np.zeros(E, bool)
    same_a[1:] = (ca[1:] == ca[:-1]) & (sa[1:] == sa[:-1])
    first_a = ~same_a

    rowa = (ca // 2) % 128
    corea = ca // 256
    cola = (ca % 2) * La + pos_a

    aux_a = np.zeros((NC, 128, 2 * La), np.float32)
    aux_a[corea[same_a], rowa[same_a], cola[same_a]] = 1.0
    place = np.full((NC, 2, 128, GH), -1, np.int16)
    fa = first_a
    place[corea[fa], (ca % 2)[fa], rowa[fa],
          (sa - (ca % 2) * GH)[fa]] = pos_a[fa].astype(np.int16)

    # perm: aux slot -> main col (per edge)
    mainpos = np.empty(E, np.int64)
    mainpos[om] = pos_m
    auxcol = np.empty(E, np.int64)
    auxcol[oa] = cola
    perm = np.full((NC, 2, 128, 2 * La), -1, np.int16)
    perm[core, mh, part % 128, auxcol] = mainpos.astype(np.int16)

    return dict(N=N, Nsh=Nsh, Nshp=NSHP, Lh=Lh, La=La,
                main_a=main_a, ends_idx=ends_idx, vplace=vplace,
                aux_a=aux_a, place=place, perm=perm)


def host_consts(inputs, x):
    W1 = np.asarray(inputs["W1"], np.float32).reshape(20)
    a_src1 = np.asarray(inputs["a_src1"], np.float32)
    a_dst1 = np.asarray(inputs["a_dst1"], np.float32)
    W2 = np.asarray(inputs["W2"], np.float32)
    a_src2 = np.asarray(inputs["a_src2"], np.float32)
    a_dst2 = np.asarray(inputs["a_dst2"], np.float32)
    c1 = float(W1 @ a_src1)
    c2 = float(W1 @ a_dst1)
    wp = np.maximum(W1, 0.0)
    wm = np.maximum(-W1, 0.0)
    qp = wp @ W2
    qm = wm @ W2
    A2 = float(qp @ a_src2)
    B2 = float(qm @ a_src2)
    C2 = float(qp @ a_dst2)
    D2 = float(qm @ a_dst2)
    xp = float(np.maximum(x, 0.0).max())
    xm = float(np.maximum(-x, 0.0).max())

    def lrelu(v):
        return v if v > 0 else 0.2 * v

    g1 = lrelu(max(c1 * xp, -c1 * xm) + max(c2 * xp, -c2 * xm))
    u2b = max(max(A2, 0.0) * xp, max(B2, 0.0) * xm)
    v2b = max(max(C2, 0.0) * xp, max(D2, 0.0) * xm)
    g2 = lrelu(u2b + v2b)
    return dict(c1=c1, c2=c2, g1=g1, A2=A2, B2=B2, C2=C2, D2=D2, g2=g2,
                qp=qp, qm=qm,
                b2=np.asarray(inputs["b2"], np.float32),
                Wl=np.asarray(inputs["Wl"], np.float32).reshape(20),
                bl=float(np.asarray(inputs["bl"], np.float32)[0]))


# ===================== bass kernel builder =====================
from contextlib import ExitStack
import concourse.bass as bass  # noqa: F401
import concourse.bacc as bacc
install()

F32 = mybir.dt.float32
BF16 = mybir.dt.bfloat16
I16 = mybir.dt.int16
AF = mybir.ActivationFunctionType
ALU = mybir.AluOpType


def build2(pp, C):
    Lh, La = pp["Lh"], pp["La"]
    L2h, L2a = 2 * Lh, 2 * La
    c1, c2, g1 = C["c1"], C["c2"], C["g1"]
    A2, B2, C2c, D2, g2 = C["A2"], C["B2"], C["C2"], C["D2"], C["g2"]
    qp, qm, b2, Wl, bl = C["qp"], C["qm"], C["b2"], C["Wl"], C["bl"]

    nc = bacc.Bacc("TRN2", target_bir_lowering=False, debug=False,
                   num_devices=8)

    def din(name, shape, dt=F32):
        return nc.dram_tensor(name, shape, dt, kind="ExternalInput")

    xg_hi = din("xg_hi", [NG, NSHP], BF16)
    xg_lo = din("xg_lo", [NG, NSHP], BF16)
    xs16_d = din("xs16", [16, DR])
    main_a_d = din("main_a", [128, L2h], BF16)
    aux_a_d = din("aux_a", [128, L2a], BF16)
    ends_d = din("ends_idx", [128, L2h], I16)
    vplace_d = din("vplace", [256, DRH], I16)
    place_d = din("place", [256, GH], I16)
    perm_d = din("perm", [256, L2a], I16)
    ind_d = din("ind", [128, 16], BF16)
    y_out = nc.dram_tensor("y", [128, 98], F32, kind="ExternalOutput")
    pcat = nc.dram_tensor("pcat", [1, 2 * NSHP], BF16)
    pcat_full = nc.dram_tensor("pcat_full", [1, 16 * NSHP], BF16,
                               addr_space="Shared")
    rp_d = nc.dram_tensor("rp_st", [1, NSHP], F32)
    rx_d = nc.dram_tensor("rx_st", [1, NSHP], F32)

    with tile.TileContext(nc) as tc, ExitStack() as ctx:
        cons = ctx.enter_context(tc.tile_pool(name="cons", bufs=1))
        pidx = ctx.enter_context(tc.tile_pool(name="pidx", bufs=1))
        datp = ctx.enter_context(tc.tile_pool(name="datp", bufs=1))
        auxp = ctx.enter_context(tc.tile_pool(name="auxp", bufs=2))
        mb = ctx.enter_context(tc.tile_pool(name="mb", bufs=6))
        mf = ctx.enter_context(tc.tile_pool(name="mf", bufs=3))
        xsr = ctx.enter_context(tc.tile_pool(name="xsr", bufs=1))
        bnd = ctx.enter_context(tc.tile_pool(name="bnd", bufs=2))
        vdp = ctx.enter_context(tc.tile_pool(name="vdp", bufs=2))
        nod = ctx.enter_context(tc.tile_pool(name="nod", bufs=6))
        fy = ctx.enter_context(tc.tile_pool(name="fy", bufs=4))
        psp = ctx.enter_context(tc.tile_pool(name="ps", bufs=2, space="PSUM"))

        main_a = cons.tile([128, L2h], BF16, name="main_a")
        nc.sync.dma_start(main_a[:], main_a_d.ap())
        aux_a = cons.tile([128, L2a], BF16, name="aux_a")
        nc.sync.dma_start(aux_a[:], aux_a_d.ap())
        ends_t = cons.tile([128, L2h], I16, name="ends_t")
        nc.sync.dma_start(ends_t[:], ends_d.ap())
        vpl = []
        for h in range(2):
            t = cons.tile([128, DRH], I16, name=f"vpl{h}")
            nc.sync.dma_start(t[:], vplace_d.ap()[128 * h:128 * (h + 1), :])
            vpl.append(t)
        ind_t = cons.tile([128, 16], BF16, name="ind")
        nc.sync.dma_start(ind_t[:], ind_d.ap())
        g1b = cons.tile([128, 1], F32, name="g1b")
        nc.vector.memset(g1b[:], -g1)
        g2b = cons.tile([128, 1], F32, name="g2b")
        nc.vector.memset(g2b[:], -g2)

        def ls(out_ap, data_ap, idx_ap, ne, ni):
            nc.gpsimd.local_scatter(out_ap, data_ap, idx_ap, channels=128,
                                    num_elems=ne, num_idxs=ni)

        def scan(out, data1, mask=None):
            m = main_a if mask is None else mask
            nc.vector.tensor_tensor_scan(out[:], m[:],
                                         data1[:], 0.0, ALU.mult, ALU.add)

        def expand_src(hi_ap_fn, lo_ap_fn, name):
            ubufs = []
            for which, apf in (("h", hi_ap_fn), ("l", lo_ap_fn)):
                ab = auxp.tile([128, L2a], BF16, tag="ab", bufs=1,
                               name=f"ab_{name}{which}")
                for h in range(2):
                    pi = pidx.tile([128, GH], I16, tag="pl", bufs=1,
                                   name=f"pl_{name}{which}{h}")
                    nc.sync.dma_start(
                        pi[:], place_d.ap()[128 * h:128 * (h + 1), :])
                    dt = datp.tile([128, GH], BF16, tag="dp", bufs=1,
                                   name=f"dp_{name}{which}{h}")
                    for g in range(NG):
                        eng = nc.scalar if g % 2 else nc.sync
                        eng.dma_start(dt[16 * g:16 * (g + 1), :], apf(g, h))
                    ls(ab[:, h * La:(h + 1) * La], dt[:], pi[:], La, GH)
                asc = auxp.tile([128, L2a], BF16, tag="as", bufs=1,
                                name=f"as_{name}{which}")
                scan(asc, ab, mask=aux_a)
                ut = mb.tile([128, L2h], BF16, tag="mb", bufs=5,
                             name=f"u{which}_{name}")
                for mh in range(2):
                    pe = pidx.tile([128, L2a], I16, tag="pe", bufs=1,
                                   name=f"pe_{name}{which}{mh}")
                    nc.sync.dma_start(
                        pe[:], perm_d.ap()[128 * mh:128 * (mh + 1), :])
                    ls(ut[:, mh * Lh:(mh + 1) * Lh], asc[:], pe[:], Lh, L2a)
                ubufs.append(ut)
            uh, ul = ubufs
            out = xsr.tile([128, L2h], F32, tag="xs", bufs=1,
                           name=f"xs_{name}")
            nc.vector.tensor_tensor(out=out[:], in0=uh[:], in1=ul[:],
                                    op=ALU.add)
            return out

        def expand_dst(v128, name):
            vh = vdp.tile([128, DR], BF16, tag="vb", bufs=2,
                          name=f"vh_{name}")
            nc.scalar.copy(vh[:], v128[:])
            vdf = vdp.tile([128, DR], F32, tag="vf", bufs=2,
                           name=f"vd_{name}")
            nc.vector.tensor_tensor(out=vdf[:], in0=v128[:], in1=vh[:],
                                    op=ALU.subtract)
            vl = vdp.tile([128, DR], BF16, tag="vb", bufs=2,
                          name=f"vl_{name}")
            nc.scalar.copy(vl[:], vdf[:])
            outs = []
            for which, vt in (("h", vh), ("l", vl)):
                st = mb.tile([128, L2h], BF16, tag="mb", bufs=5,
                             name=f"vs_{name}{which}")
                for h in range(2):
                    ls(st[:, h * Lh:(h + 1) * Lh],
                       vt[:, h * DRH:(h + 1) * DRH], vpl[h][:], Lh, DRH)
                vb = mb.tile([128, L2h], BF16, tag="mb", bufs=5,
                             name=f"vb_{name}{which}")
                scan(vb, st)
                outs.append(vb)
            return outs

        def boundary_hilo(s, dest, name):
            shi = mb.tile([128, L2h], BF16, tag="sb", bufs=2,
                          name=f"shi_{name}")
            nc.scalar.copy(shi[:], s[:])
            sdf = mf.tile([128, L2h], F32, tag="mf", bufs=3,
                          name=f"sdf_{name}")
            nc.vector.tensor_tensor(out=sdf[:], in0=s[:], in1=shi[:],
                                    op=ALU.subtract)
            slo = mb.tile([128, L2h], BF16, tag="sb", bufs=2,
                          name=f"slo_{name}")
            nc.scalar.copy(slo[:], sdf[:])
            bh = bnd.tile([128, DR], BF16, tag="bd", bufs=2,
                          name=f"bh_{name}")
            ls(bh[:], shi[:], ends_t[:], DR, L2h)
            bl = bnd.tile([128, DR], BF16, tag="bd", bufs=2,
                          name=f"bl_{name}")
            ls(bl[:], slo[:], ends_t[:], DR, L2h)
            for h in range(2):
                ps = psp.tile([16, DRH], F32, tag="ps", name=f"ps_{name}{h}")
                nc.tensor.matmul(ps[:], ind_t[:],
                                 bh[:, h * DRH:(h + 1) * DRH],
                                 start=True, stop=False)
                nc.tensor.matmul(ps[:], ind_t[:],
                                 bl[:, h * DRH:(h + 1) * DRH],
                                 start=False, stop=True)
                nc.scalar.copy(dest[:, h * DRH:(h + 1) * DRH], ps[:])

        def boundary_bf(t, dest, name):
            b = bnd.tile([128, DR], BF16, tag="bd", bufs=2, name=f"b_{name}")
            ls(b[:], t[:], ends_t[:], DR, L2h)
            for h in range(2):
                ps = psp.tile([16, DRH], F32, tag="ps", name=f"pb_{name}{h}")
                nc.tensor.matmul(ps[:], ind_t[:],
                                 b[:, h * DRH:(h + 1) * DRH],
                                 start=True, stop=True)
                nc.scalar.copy(dest[:, h * DRH:(h + 1) * DRH], ps[:])

        # ================== LAYER 1 ==================
        xsrc = expand_src(
            lambda g, h: xg_hi.ap()[g:g + 1, h * GH:(h + 1) * GH]
            .partition_broadcast(16),
            lambda g, h: xg_lo.ap()[g:g + 1, h * GH:(h + 1) * GH]
            .partition_broadcast(16), "x")

        xs16t = nod.tile([16, DR], F32, tag="nd", bufs=6, name="xs16t")
        nc.sync.dma_start(xs16t[:], xs16_d.ap())
        vdraw = vdp.tile([128, DR], F32, tag="vf", bufs=2, name="vdraw")
        for g in range(NG):
            eng = nc.scalar if g % 2 else nc.sync
            eng.dma_start(vdraw[16 * g:16 * (g + 1), :], xs16_d.ap())
        nc.vector.tensor_scalar(out=vdraw[:], in0=vdraw[:], scalar1=c2,
                                scalar2=None, op0=ALU.mult)
        vbh, vbl = expand_dst(vdraw, "v1")

        t1f = mf.tile([128, L2h], F32, tag="mf", bufs=3, name="t1f")
        nc.vector.scalar_tensor_tensor(out=t1f[:], in0=xsrc[:], scalar=c1,
                                       in1=vbh[:], op0=ALU.mult, op1=ALU.add)
        epre = mf.tile([128, L2h], F32, tag="mf", bufs=3, name="epre")
        nc.vector.tensor_tensor(out=epre[:], in0=t1f[:], in1=vbl[:],
                                op=ALU.add)
        nc.vector.scalar_tensor_tensor(out=epre[:], in0=epre[:], scalar=0.2,
                                       in1=epre[:], op0=ALU.mult, op1=ALU.max)
        numer = mf.tile([128, L2h], F32, tag="mf", bufs=3, name="numer")
        nc.scalar.activation(numer[:], epre[:], AF.Exp, bias=g1b[:])
        s0 = mf.tile([128, L2h], F32, tag="mf", bufs=3, name="s0")
        scan(s0, numer)
        w1 = mf.tile([128, L2h], F32, tag="mf", bufs=3, name="w1")
        nc.vector.tensor_tensor(out=w1[:], in0=numer[:], in1=xsrc[:],
                                op=ALU.mult)
        s1 = mf.tile([128, L2h], F32, tag="mf", bufs=3, name="s1")
        scan(s1, w1)

        den1 = nod.tile([16, DR], F32, tag="nd", bufs=6, name="den1")
        P1 = nod.tile([16, DR], F32, tag="nd", bufs=6, name="P1")
        boundary_hilo(s0, den1, "d1")
        boundary_hilo(s1, P1, "p1")

        est = nod.tile([16, DR], F32, tag="nd", bufs=6, name="est")
        nc.vector.tensor_scalar(out=est[:], in0=xs16t[:], scalar1=c1 + c2,
                                scalar2=None, op0=ALU.mult)
        nc.vector.scalar_tensor_tensor(out=est[:], in0=est[:], scalar=0.2,
                                       in1=est[:], op0=ALU.mult, op1=ALU.max)
        dens = nod.tile([16, DR], F32, tag="nd", bufs=6, name="dens")
        nc.scalar.activation(dens[:], est[:], AF.Exp, bias=g1b[0:16, :])
        nc.vector.tensor_tensor(out=den1[:], in0=den1[:], in1=dens[:],
                                op=ALU.add)
        nc.vector.tensor_tensor(out=est[:], in0=dens[:], in1=xs16t[:],
                                op=ALU.mult)
        nc.vector.tensor_tensor(out=P1[:], in0=P1[:], in1=est[:],
                                op=ALU.add)
        nc.vector.tensor_scalar(out=den1[:], in0=den1[:], scalar1=1e-30,
                                scalar2=None, op0=ALU.add)
        rec1 = nod.tile([16, DR], F32, tag="nd", bufs=6, name="rec1")
        nc.vector.reciprocal(rec1[:], den1[:])
        Pn = nod.tile([16, DR], F32, tag="pn", bufs=1, name="Pn")
        nc.vector.tensor_tensor(out=Pn[:], in0=P1[:], in1=rec1[:],
                                op=ALU.mult)

        phi = nod.tile([16, DR], BF16, tag="nb", bufs=2, name="phi")
        nc.scalar.copy(phi[:], Pn[:])
        pdf = nod.tile([16, DR], F32, tag="nd", bufs=6, name="pdf")
        nc.vector.tensor_tensor(out=pdf[:], in0=Pn[:], in1=phi[:],
                                op=ALU.subtract)
        plo = nod.tile([16, DR], BF16, tag="nb", bufs=2, name="plo")
        nc.scalar.copy(plo[:], pdf[:])
        nc.sync.dma_start(pcat.ap()[:, 0:NSHP], phi[:])
        nc.sync.dma_start(pcat.ap()[:, NSHP:2 * NSHP], plo[:])
        nc.gpsimd.collective_compute(
            "AllGather", ALU.bypass, replica_groups=[list(range(8))],
            ins=[pcat.ap()], outs=[pcat_full.ap()])

        # ================== LAYER 2 ==================
        p2 = expand_src(
            lambda g, h: pcat_full.ap()[:, g * 2 * NSHP + h * GH:
                                        g * 2 * NSHP + (h + 1) * GH]
            .partition_broadcast(16),
            lambda g, h: pcat_full.ap()[:, g * 2 * NSHP + NSHP + h * GH:
                                        g * 2 * NSHP + NSHP + (h + 1) * GH]
            .partition_broadcast(16), "p")

        pn128 = vdp.tile([128, DR], F32, tag="vf", bufs=2, name="pn128")
        for g in range(NG):
            eng = nc.scalar if g % 2 else nc.sync
            eng.dma_start(pn128[16 * g:16 * (g + 1), :], Pn[:])
        v2dat = vdp.tile([128, DR], F32, tag="vf", bufs=2, name="v2dat")
        nc.vector.tensor_scalar(out=v2dat[:], in0=pn128[:], scalar1=0.0,
                                scalar2=C2c + D2, op0=ALU.max, op1=ALU.mult)
        nc.vector.scalar_tensor_tensor(out=v2dat[:], in0=pn128[:],
                                       scalar=-D2, in1=v2dat[:],
                                       op0=ALU.mult, op1=ALU.add)
        v2bh, v2bl = expand_dst(v2dat, "v2")

        u2t = mf.tile([128, L2h], F32, tag="mf", bufs=3, name="u2t")
        nc.vector.tensor_scalar(out=u2t[:], in0=p2[:], scalar1=0.0,
                                scalar2=A2 + B2, op0=ALU.max, op1=ALU.mult)
        ep2 = mf.tile([128, L2h], F32, tag="mf", bufs=3, name="ep2")
        nc.vector.scalar_tensor_tensor(out=ep2[:], in0=p2[:], scalar=-B2,
                                       in1=u2t[:], op0=ALU.mult, op1=ALU.add)
        nc.vector.tensor_tensor(out=ep2[:], in0=ep2[:], in1=v2bh[:],
                                op=ALU.add)
        nc.vector.tensor_tensor(out=ep2[:], in0=ep2[:], in1=v2bl[:],
                                op=ALU.add)
        nc.vector.scalar_tensor_tensor(out=ep2[:], in0=ep2[:], scalar=0.2,
                                       in1=ep2[:], op0=ALU.mult, op1=ALU.max)
        numer2 = mb.tile([128, L2h], BF16, tag="mb", bufs=5, name="numer2")
        nc.scalar.activation(numer2[:], ep2[:], AF.Exp, bias=g2b[:])
        t0 = mb.tile([128, L2h], BF16, tag="mb", bufs=5, name="t0")
        scan(t0, numer2)
        w21 = mb.tile([128, L2h], BF16, tag="mb", bufs=5, name="w21")
        nc.vector.scalar_tensor_tensor(out=w21[:], in0=p2[:], scalar=0.0,
                                       in1=numer2[:], op0=ALU.max,
                                       op1=ALU.mult)
        t1 = mb.tile([128, L2h], BF16, tag="mb", bufs=5, name="t1")
        scan(t1, w21)
        w2x = mb.tile([128, L2h], BF16, tag="mb", bufs=5, name="w2x")
        nc.vector.tensor_tensor(out=w2x[:], in0=numer2[:], in1=p2[:],
                                op=ALU.mult)
        t2 = mb.tile([128, L2h], BF16, tag="mb", bufs=5, name="t2")
        scan(t2, w2x)

        den2 = nod.tile([16, DR], F32, tag="nd", bufs=6, name="den2")
        Sp = nod.tile([16, DR], F32, tag="nd", bufs=6, name="Sp")
        Sx = nod.tile([16, DR], F32, tag="nd", bufs=6, name="Sx")
        boundary_bf(t0, den2, "d2")
        boundary_bf(t1, Sp, "sp")
        boundary_bf(t2, Sx, "sx")

        rpn = nod.tile([16, DR], F32, tag="nd", bufs=6, name="rpn")
        nc.vector.tensor_scalar(out=rpn[:], in0=Pn[:], scalar1=0.0,
                                scalar2=None, op0=ALU.max)
        e2s = nod.tile([16, DR], F32, tag="nd", bufs=6, name="e2s")
        nc.vector.tensor_scalar(out=e2s[:], in0=rpn[:],
                                scalar1=A2 + B2 + C2c + D2,
                                scalar2=None, op0=ALU.mult)
        nc.vector.scalar_tensor_tensor(out=e2s[:], in0=Pn[:],
                                       scalar=-(B2 + D2), in1=e2s[:],
                                       op0=ALU.mult, op1=ALU.add)
        nc.vector.scalar_tensor_tensor(out=e2s[:], in0=e2s[:], scalar=0.2,
                                       in1=e2s[:], op0=ALU.mult, op1=ALU.max)
        d2s = nod.tile([16, DR], F32, tag="nd", bufs=6, name="d2s")
        nc.scalar.activation(d2s[:], e2s[:], AF.Exp, bias=g2b[0:16, :])
        nc.vector.tensor_tensor(out=den2[:], in0=den2[:], in1=d2s[:],
                                op=ALU.add)
        nc.vector.tensor_tensor(out=e2s[:], in0=d2s[:], in1=rpn[:],
                                op=ALU.mult)
        nc.vector.tensor_tensor(out=Sp[:], in0=Sp[:], in1=e2s[:],
                                op=ALU.add)
        nc.vector.tensor_tensor(out=e2s[:], in0=d2s[:], in1=Pn[:],
                                op=ALU.mult)
        nc.vector.tensor_tensor(out=Sx[:], in0=Sx[:], in1=e2s[:],
                                op=ALU.add)

        nc.vector.tensor_scalar(out=den2[:], in0=den2[:], scalar1=1e-30,
                                scalar2=None, op0=ALU.add)
        rec2 = nod.tile([16, DR], F32, tag="nd", bufs=6, name="rec2")
        nc.vector.reciprocal(rec2[:], den2[:])
        nc.vector.tensor_tensor(out=Sp[:], in0=Sp[:], in1=rec2[:],
                                op=ALU.mult)
        nc.vector.tensor_tensor(out=Sx[:], in0=Sx[:], in1=rec2[:],
                                op=ALU.mult)
        nc.sync.dma_start(rp_d.ap(), Sp[:])
        nc.sync.dma_start(rx_d.ap(), Sx[:])

        rp128 = fy.tile([128, 98], F32, tag="fy", bufs=3, name="rp128")
        nc.sync.dma_start(rp128[:], rp_d.ap())
        rm128 = fy.tile([128, 98], F32, tag="fy", bufs=3, name="rm128")
        nc.sync.dma_start(rm128[:], rx_d.ap())
        nc.vector.tensor_tensor(out=rm128[:], in0=rp128[:], in1=rm128[:],
                                op=ALU.subtract)
        yacc = fy.tile([128, 98], F32, tag="fy", bufs=3, name="yacc")
        nc.vector.memset(yacc[:], float(bl))
        for k in range(20):
            tk = fy.tile([128, 98], F32, tag="yk", bufs=2, name=f"tk{k}")
            nc.vector.tensor_scalar(out=tk[:], in0=rp128[:],
                                    scalar1=float(qp[k]),
                                    scalar2=float(b2[k]), op0=ALU.mult,
                                    op1=ALU.add)
            nc.vector.scalar_tensor_tensor(out=tk[:], in0=rm128[:],
                                           scalar=float(qm[k]), in1=tk[:],
                                           op0=ALU.mult, op1=ALU.add)
            hk = fy.tile([128, 98], F32, tag="yk", bufs=2, name=f"hk{k}")
            nc.scalar.activation(hk[:], tk[:], AF.Relu)
            nc.vector.scalar_tensor_tensor(out=yacc[:], in0=hk[:],
                                           scalar=float(Wl[k]), in1=yacc[:],
                                           op0=ALU.mult, op1=ALU.add)
        nc.sync.dma_start(y_out.ap(), yacc[:])

    nc.compile()
    return nc


def make_in_maps2(pp, inputs):
    x = np.asarray(inputs["x"], np.float32).reshape(-1)
    Nsh = pp["Nsh"]
    xg = np.zeros((NG, NSHP), np.float32)
    for g in range(NG):
        xg[g, :Nsh] = x[g * Nsh:(g + 1) * Nsh]
    xg_hi = np.asarray(xg, BF)
    xg_lo = np.asarray(xg - xg_hi.astype(np.float32), BF)
    ind = np.zeros((128, 16), BF)
    for p in range(128):
        ind[p, p % 16] = 1.0
    maps = []
    for c in range(NC):
        xs16 = np.zeros((16, DR), np.float32)
        xs16.reshape(-1)[:Nsh] = x[c * Nsh:(c + 1) * Nsh]
        maps.append({
            "xg_hi": xg_hi, "xg_lo": xg_lo, "ind": ind, "xs16": xs16,
            "main_a": np.asarray(pp["main_a"][c], BF),
            "aux_a": np.asarray(pp["aux_a"][c], BF),
            "ends_idx": pp["ends_idx"][c],
            "vplace": pp["vplace"][c].reshape(256, DRH),
            "place": pp["place"][c].reshape(256, GH),
            "perm": pp["perm"][c].reshape(256, 2 * pp["La"]),
        })
    return maps


# ===================== runner =====================

def _run_spmd(nc, maps):
    from concourse.bass_utils import run_bass_kernel_spmd
    return run_bass_kernel_spmd(nc, maps, list(range(8)))


def kernel(**inputs):
    x = np.asarray(inputs["x"], np.float32)
    N = x.shape[0]
    if np.any(np.asarray(inputs["b1"])) or N % NC or (N // NC) > NSHP:
        return _kernel_numpy(**inputs)
    pp = prep2(np.asarray(inputs["edge_index"]), N)
    C = host_consts(inputs, x.reshape(-1))
    nc = build2(pp, C)
    maps = make_in_maps2(pp, inputs)
    res = _run_spmd(nc, maps)
    Nsh = pp["Nsh"]
    y = np.zeros((N, 1), np.float32)
    for c in range(NC):
        y[c * Nsh:(c + 1) * Nsh, 0] = res.results[c]["y"].reshape(-1)[:Nsh]
    return y


def _kernel_numpy(x, edge_index, W1, a_src1, a_dst1, b1, W2, a_src2, a_dst2,
                  b2, Wl, bl):
    def lr(v):
        return np.where(v > 0, v, 0.2 * v).astype(np.float32)

    def conv(h, src, dst, W, asrc, adst, b, n):
        hh = (h @ W).astype(np.float32)
        u, v = hh @ asrc, hh @ adst
        e = lr(u[src] + v[dst])
        m = np.full(n, -np.inf, np.float32)
        np.maximum.at(m, dst, e)
        ee = np.exp(e - m[dst]).astype(np.float32)
        den = np.bincount(dst, weights=ee, minlength=n).astype(np.float32)
        al = ee / (den[dst] + 1e-16)
        out = np.zeros((n, hh.shape[1]), np.float32)
        wh = hh[src] * al[:, None]
        for k in range(hh.shape[1]):
            out[:, k] = np.bincount(dst, weights=wh[:, k], minlength=n)
        return out + b

    n = x.shape[0]
    loop = np.arange(n, dtype=np.int64)
    src = np.concatenate([edge_index[0], loop])
    dst = np.concatenate([edge_index[1], loop])
    h = np.maximum(conv(np.asarray(x, np.float32), src, dst, W1, a_src1,
                        a_dst1, b1, n), 0)
    h = np.maximum(conv(h, src, dst, W2, a_src2, a_dst2, b2, n), 0)
    return (h @ Wl + bl).astype(np.float32)


# revision 5
# speedup vs baseline: 1.9126x; 1.9126x over previous
"""Self-contained Trainium2 Bass kernel for the 2-layer GAT problem.

Accepts FULL inputs (as produced by setup_inputs()), shards across the
8 NeuronCores internally (dst-sharded edges), returns the full
[100000, 1] float32 output.

Design: host index preprocessing builds, per core, per-partition edge
streams (partition = (src-group, dst-range), dst-sorted) plus static
local_scatter index maps. On device, node->edge-stream expansion is
done with gpsimd local_scatter (per-partition indices, bf16 values) +
DVE segmented-broadcast scans; per-dst softmax sums are extracted by
scattering scan values at run-ends and reducing over the 8 src groups
with an indicator matmul on the PE. Self-loops are handled
analytically as node-wise terms. All weight-derived scalars are baked
into the program at build time.

Falls back to a numpy reference path if b1 != 0 or N % 8 != 0.
"""
import numpy as np
import ml_dtypes

# ===================== tile drain workaround =====================
"""This walrus build allows at most 1 sync-wait on SP CTRL instructions,
but TileContext's tail drain aggregates all end-of-kernel waits onto one
Drain. Split them across nops instead."""
import concourse.tile as tile
from concourse import mybir
from bass_rust import ScopedClock


def _patched_drain_and_barrier(self, tick_clock, wait_clock):
    nc = self.nc
    probe = nc.sync.nop()
    wait_clock.add_sem_waits(probe.ins, ScopedClock({None: tick_clock.global_clock}))
    si = probe.ins.sync_info
    waits = list(si.on_wait) if si is not None else []
    if si is not None:
        si.on_wait = waits[:1]
    for w in waits[1:]:
        nop = nc.sync.nop()
        nop.ins.sync_info = mybir.SyncInfo(on_wait=[w], on_update=[])
    nc.sync.drain()
    nc.all_engine_barrier()
    popped = nc._tile_sem_poison_stack.pop()
    assert popped is self._sem_poison
    nc.clear_and_free_semaphores(list(self.sems.allocated().values()))
    nc.all_engine_barrier()


def install():
    tile.TileContext._drain_and_barrier = _patched_drain_and_barrier


# ===================== host preprocessing =====================
BF = ml_dtypes.bfloat16
NC = 8      # cores (dst shards)
NG = 8      # src groups
NSC = 16    # dst ranges per core
DR = 784    # dsts per range (12544 padded shard)
DRH = 392
GH = 6272   # src-group half size (padded)
NSHP = NSC * DR


def _ceil16(v):
    return int(-(-int(v) // 16) * 16)


def prep2(edge_index, N):
    Nsh = N // NC
    src = np.asarray(edge_index[0], np.int64)
    dst = np.asarray(edge_index[1], np.int64)
    E = src.shape[0]
    core = dst // Nsh
    g = src // Nsh
    dl = (dst - core * Nsh).astype(np.int64)
    sl = (src - g * Nsh).astype(np.int64)
    part = g * NSC + dl // DR
    mh = (dl % DR) // DRH
    ah = sl // GH
    cellm = (core * 128 + part) * 2 + mh
    cella = (core * 128 + part) * 2 + ah

    # main (dst-sorted) streams
    om = np.lexsort((sl, dl, cellm))
    cm, dm = cellm[om], dl[om]
    counts_m = np.bincount(cm, minlength=NC * 256)
    Lh = _ceil16(counts_m.max() + 1)
    assert Lh <= 2046, f"Lh={Lh} exceeds local_scatter limit"
    starts_m = np.zeros(NC * 256, np.int64)
    np.cumsum(counts_m[:-1], out=starts_m[1:])
    pos_m = np.arange(E, dtype=np.int64) - starts_m[cm]
    same_m = np.zeros(E, bool)
    same_m[1:] = (cm[1:] == cm[:-1]) & (dm[1:] == dm[:-1])
    first_m = ~same_m
    last_m = np.ones(E, bool)
    last_m[:-1] = ~same_m[1:]

    rowm = (cm // 2) % 128
    corem = cm // 256
    colm = (cm % 2) * Lh + pos_m

    main_a = np.zeros((NC, 128, 2 * Lh), np.float32)
    main_a[corem[same_m], rowm[same_m], colm[same_m]] = 1.0
    ends_idx = np.full((NC, 128, 2 * Lh), -1, np.int16)
    ends_idx[corem[last_m], rowm[last_m], colm[last_m]] = (
        dm[last_m] % DR).astype(np.int16)
    vplace = np.full((NC, 2, 128, DRH), -1, np.int16)
    fm = first_m
    vplace[corem[fm], (cm % 2)[fm], rowm[fm],
           ((dm % DR) % DRH)[fm]] = pos_m[fm].astype(np.int16)

    # aux (src-sorted) streams
    oa = np.lexsort((dl, sl, cella))
    ca, sa = cella[oa], sl[oa]
    counts_a = np.bincount(ca, minlength=NC * 256)
    La = _ceil16(counts_a.max() + 1)
    assert La <= 2046, f"La={La}"
    starts_a = np.zeros(NC * 256, np.int64)
    np.cumsum(counts_a[:-1], out=starts_a[1:])
    pos_a = np.arange(E, dtype=np.int64) - starts_a[ca]
    same_a = np.zeros(E, bool)
    same_a[1:] = (ca[1:] == ca[:-1]) & (sa[1:] == sa[:-1])
    first_a = ~same_a

    rowa = (ca // 2) % 128
    corea = ca // 256
    cola = (ca % 2) * La + pos_a

    aux_a = np.zeros((NC, 128, 2 * La), np.float32)
    aux_a[corea[same_a], rowa[same_a], cola[same_a]] = 1.0
    place = np.full((NC, 2, 128, GH), -1, np.int16)
    fa = first_a
    place[corea[fa], (ca % 2)[fa], rowa[fa],
          (sa - (ca % 2) * GH)[fa]] = pos_a[fa].astype(np.int16)

    # perm: aux slot -> main col (per edge)
    mainpos = np.empty(E, np.int64)
    mainpos[om] = pos_m
    auxcol = np.empty(E, np.int64)
    auxcol[oa] = cola
    perm = np.full((NC, 2, 128, 2 * La), -1, np.int16)
    perm[core, mh, part % 128, auxcol] = mainpos.astype(np.int16)

    return dict(N=N, Nsh=Nsh, Nshp=NSHP, Lh=Lh, La=La,
                main_a=main_a, ends_idx=ends_idx, vplace=vplace,
                aux_a=aux_a, place=place, perm=perm)


def host_consts(inputs, x):
    W1 = np.asarray(inputs["W1"], np.float32).reshape(20)
    a_src1 = np.asarray(inputs["a_src1"], np.float32)
    a_dst1 = np.asarray(inputs["a_dst1"], np.float32)
    W2 = np.asarray(inputs["W2"], np.float32)
    a_src2 = np.asarray(inputs["a_src2"], np.float32)
    a_dst2 = np.asarray(inputs["a_dst2"], np.float32)
    c1 = float(W1 @ a_src1)
    c2 = float(W1 @ a_dst1)
    wp = np.maximum(W1, 0.0)
    wm = np.maximum(-W1, 0.0)
    qp = wp @ W2
    qm = wm @ W2
    A2 = float(qp @ a_src2)
    B2 = float(qm @ a_src2)
    C2 = float(qp @ a_dst2)
    D2 = float(qm @ a_dst2)
    xp = float(np.maximum(x, 0.0).max())
    xm = float(np.maximum(-x, 0.0).max())

    def lrelu(v):
        return v if v > 0 else 0.2 * v

    g1 = lrelu(max(c1 * xp, -c1 * xm) + max(c2 * xp, -c2 * xm))
    u2b = max(max(A2, 0.0) * xp, max(B2, 0.0) * xm)
    v2b = max(max(C2, 0.0) * xp, max(D2, 0.0) * xm)
    g2 = lrelu(u2b + v2b)
    return dict(c1=c1, c2=c2, g1=g1, A2=A2, B2=B2, C2=C2, D2=D2, g2=g2,
                qp=qp, qm=qm,
                b2=np.asarray(inputs["b2"], np.float32),
                Wl=np.asarray(inputs["Wl"], np.float32).reshape(20),
                bl=float(np.asarray(inputs["bl"], np.float32)[0]))


# ===================== bass kernel builder =====================
from contextlib import ExitStack
import concourse.bass as bass  # noqa: F401
import concourse.bacc as bacc
install()

F32 = mybir.dt.float32
BF16 = mybir.dt.bfloat16
I16 = mybir.dt.int16
AF = mybir.ActivationFunctionType
ALU = mybir.AluOpType


def build2(pp, C):
    Lh, La = pp["Lh"], pp["La"]
    L2h, L2a = 2 * Lh, 2 * La
    c1, c2, g1 = C["c1"], C["c2"], C["g1"]
    A2, B2, C2c, D2, g2 = C["A2"], C["B2"], C["C2"], C["D2"], C["g2"]
    qp, qm, b2, Wl, bl = C["qp"], C["qm"], C["b2"], C["Wl"], C["bl"]

    nc = bacc.Bacc("TRN2", target_bir_lowering=False, debug=False,
                   num_devices=8)

    def din(name, shape, dt=F32):
        return nc.dram_tensor(name, shape, dt, kind="ExternalInput")

    xg_hi = din("xg_hi", [NG, NSHP], BF16)
    xs16_d = din("xs16", [16, DR])
    main_a_d = din("main_a", [128, L2h], BF16)
    aux_a_d = din("aux_a", [128, L2a], BF16)
    ends_d = din("ends_idx", [128, L2h], I16)
    vplace_d = din("vplace", [256, DRH], I16)
    place_d = din("place", [256, GH], I16)
    perm_d = din("perm", [256, L2a], I16)
    ind_d = din("ind", [128, 16], BF16)
    y_out = nc.dram_tensor("y", [128, 98], F32, kind="ExternalOutput")
    pcat = nc.dram_tensor("pcat", [1, NSHP], BF16)
    pcat_full = nc.dram_tensor("pcat_full", [1, 8 * NSHP], BF16,
                               addr_space="Shared")
    rp_d = nc.dram_tensor("rp_st", [1, NSHP], F32)
    rx_d = nc.dram_tensor("rx_st", [1, NSHP], F32)

    with tile.TileContext(nc) as tc, ExitStack() as ctx:
        cons = ctx.enter_context(tc.tile_pool(name="cons", bufs=1))
        pidx = ctx.enter_context(tc.tile_pool(name="pidx", bufs=2))
        datp = ctx.enter_context(tc.tile_pool(name="datp", bufs=2))
        auxp = ctx.enter_context(tc.tile_pool(name="auxp", bufs=2))
        mb = ctx.enter_context(tc.tile_pool(name="mb", bufs=6))
        mf = ctx.enter_context(tc.tile_pool(name="mf", bufs=3))
        bnd = ctx.enter_context(tc.tile_pool(name="bnd", bufs=3))
        vdp = ctx.enter_context(tc.tile_pool(name="vdp", bufs=2))
        nod = ctx.enter_context(tc.tile_pool(name="nod", bufs=6))
        fy = ctx.enter_context(tc.tile_pool(name="fy", bufs=4))
        psp = ctx.enter_context(tc.tile_pool(name="ps", bufs=4, space="PSUM"))

        main_a = cons.tile([128, L2h], BF16, name="main_a")
        nc.sync.dma_start(main_a[:], main_a_d.ap())
        aux_a = cons.tile([128, L2a], BF16, name="aux_a")
        nc.sync.dma_start(aux_a[:], aux_a_d.ap())
        ends_t = cons.tile([128, L2h], I16, name="ends_t")
        nc.sync.dma_start(ends_t[:], ends_d.ap())
        vpl = []
        for h in range(2):
            t = cons.tile([128, DRH], I16, name=f"vpl{h}")
            nc.sync.dma_start(t[:], vplace_d.ap()[128 * h:128 * (h + 1), :])
            vpl.append(t)
        pls = []
        for h in range(2):
            t = cons.tile([128, GH], I16, name=f"pls{h}")
            nc.sync.dma_start(t[:], place_d.ap()[128 * h:128 * (h + 1), :])
            pls.append(t)
        ind_t = cons.tile([128, 16], BF16, name="ind")
        nc.sync.dma_start(ind_t[:], ind_d.ap())
        g1b = cons.tile([128, 1], F32, name="g1b")
        nc.vector.memset(g1b[:], -g1)
        g2b = cons.tile([128, 1], F32, name="g2b")
        nc.vector.memset(g2b[:], -g2)

        def ls(out_ap, data_ap, idx_ap, ne, ni):
            nc.gpsimd.local_scatter(out_ap, data_ap, idx_ap, channels=128,
                                    num_elems=ne, num_idxs=ni)

        def scan(out, data1, mask=None):
            m = main_a if mask is None else mask
            nc.vector.tensor_tensor_scan(out[:], m[:],
                                         data1[:], 0.0, ALU.mult, ALU.add)

        def expand_src(ap_fn, name):
            """bf16 node data (per group slice APs) -> bf16 [128, L2h]."""
            ab = auxp.tile([128, L2a], BF16, tag="ab", bufs=1,
                           name=f"ab_{name}")
            for h in range(2):
                dt = datp.tile([128, GH], BF16, tag="dp", bufs=2,
                               name=f"dp_{name}{h}")
                for g in range(NG):
                    eng = nc.scalar if g % 2 else nc.sync
                    eng.dma_start(dt[16 * g:16 * (g + 1), :], ap_fn(g, h))
                ls(ab[:, h * La:(h + 1) * La], dt[:], pls[h][:], La, GH)
            asc = auxp.tile([128, L2a], BF16, tag="as", bufs=1,
                            name=f"as_{name}")
            scan(asc, ab, mask=aux_a)
            ut = mb.tile([128, L2h], BF16, tag="mb", bufs=6,
                         name=f"u_{name}")
            for mh in range(2):
                pe = pidx.tile([128, L2a], I16, tag="pe", bufs=1,
                               name=f"pe_{name}{mh}")
                nc.sync.dma_start(
                    pe[:], perm_d.ap()[128 * mh:128 * (mh + 1), :])
                ls(ut[:, mh * Lh:(mh + 1) * Lh], asc[:], pe[:], Lh, L2a)
            return ut

        def expand_dst(v128, name):
            """[128, DR] f32 -> bf16 [128, L2h] run-constant stream."""
            vh = vdp.tile([128, DR], BF16, tag="vb", bufs=2,
                          name=f"vh_{name}")
            nc.scalar.copy(vh[:], v128[:])
            st = mb.tile([128, L2h], BF16, tag="mb", bufs=6,
                         name=f"vs_{name}")
            for h in range(2):
                ls(st[:, h * Lh:(h + 1) * Lh],
                   vh[:, h * DRH:(h + 1) * DRH], vpl[h][:], Lh, DRH)
            vb = mb.tile([128, L2h], BF16, tag="mb", bufs=6,
                         name=f"vb_{name}")
            scan(vb, st)
            return vb

        def boundary_bf(t, dest, name):
            b = bnd.tile([128, DR], BF16, tag="bd", bufs=3, name=f"b_{name}")
            ls(b[:], t[:], ends_t[:], DR, L2h)
            for h in range(2):
                ps = psp.tile([16, DRH], F32, tag="ps", name=f"pb_{name}{h}")
                nc.tensor.matmul(ps[:], ind_t[:],
                                 b[:, h * DRH:(h + 1) * DRH],
                                 start=True, stop=True)
                nc.scalar.copy(dest[:, h * DRH:(h + 1) * DRH], ps[:])

        # ================== LAYER 1 ==================
        xsrc = expand_src(
            lambda g, h: xg_hi.ap()[g:g + 1, h * GH:(h + 1) * GH]
            .partition_broadcast(16), "x")

        xs16t = nod.tile([16, DR], F32, tag="nd", bufs=6, name="xs16t")
        nc.sync.dma_start(xs16t[:], xs16_d.ap())
        vdraw = vdp.tile([128, DR], F32, tag="vf", bufs=2, name="vdraw")
        for g in range(NG):
            eng = nc.scalar if g % 2 else nc.sync
            eng.dma_start(vdraw[16 * g:16 * (g + 1), :], xs16_d.ap())
        nc.vector.tensor_scalar(out=vdraw[:], in0=vdraw[:], scalar1=c2,
                                scalar2=None, op0=ALU.mult)
        vb1 = expand_dst(vdraw, "v1")

        epre = mf.tile([128, L2h], F32, tag="mf", bufs=2, name="epre")
        nc.vector.scalar_tensor_tensor(out=epre[:], in0=xsrc[:], scalar=c1,
                                       in1=vb1[:], op0=ALU.mult, op1=ALU.add)
        nc.vector.scalar_tensor_tensor(out=epre[:], in0=epre[:], scalar=0.2,
                                       in1=epre[:], op0=ALU.mult, op1=ALU.max)
        numer = mf.tile([128, L2h], F32, tag="mf", bufs=2, name="numer")
        nc.scalar.activation(numer[:], epre[:], AF.Exp, bias=g1b[:])
        s0 = mb.tile([128, L2h], BF16, tag="mb", bufs=6, name="s0")
        scan(s0, numer)
        w1 = mf.tile([128, L2h], F32, tag="mf", bufs=2, name="w1")
        nc.vector.tensor_tensor(out=w1[:], in0=numer[:], in1=xsrc[:],
                                op=ALU.mult)
        s1 = mb.tile([128, L2h], BF16, tag="mb", bufs=6, name="s1")
        scan(s1, w1)

        den1 = nod.tile([16, DR], F32, tag="nd", bufs=6, name="den1")
        P1 = nod.tile([16, DR], F32, tag="nd", bufs=6, name="P1")
        boundary_bf(s0, den1, "d1")
        boundary_bf(s1, P1, "p1")

        est = nod.tile([16, DR], F32, tag="nd", bufs=6, name="est")
        nc.vector.tensor_scalar(out=est[:], in0=xs16t[:], scalar1=c1 + c2,
                                scalar2=None, op0=ALU.mult)
        nc.vector.scalar_tensor_tensor(out=est[:], in0=est[:], scalar=0.2,
                                       in1=est[:], op0=ALU.mult, op1=ALU.max)
        dens = nod.tile([16, DR], F32, tag="nd", bufs=6, name="dens")
        nc.scalar.activation(dens[:], est[:], AF.Exp, bias=g1b[0:16, :])
        nc.vector.tensor_tensor(out=den1[:], in0=den1[:], in1=dens[:],
                                op=ALU.add)
        nc.vector.tensor_tensor(out=est[:], in0=dens[:], in1=xs16t[:],
                                op=ALU.mult)
        nc.vector.tensor_tensor(out=P1[:], in0=P1[:], in1=est[:],
                                op=ALU.add)
        nc.vector.tensor_scalar(out=den1[:], in0=den1[:], scalar1=1e-30,
                                scalar2=None, op0=ALU.add)
        rec1 = nod.tile([16, DR], F32, tag="nd", bufs=6, name="rec1")
        nc.vector.reciprocal(rec1[:], den1[:])
        Pn = nod.tile([16, DR], F32, tag="pn", bufs=1, name="Pn")
        nc.vector.tensor_tensor(out=Pn[:], in0=P1[:], in1=rec1[:],
                                op=ALU.mult)

        phi = nod.tile([16, DR], BF16, tag="nb", bufs=2, name="phi")
        nc.scalar.copy(phi[:], Pn[:])
        nc.sync.dma_start(pcat.ap(), phi[:])
        nc.gpsimd.collective_compute(
            "AllGather", ALU.bypass, replica_groups=[list(range(8))],
            ins=[pcat.ap()], outs=[pcat_full.ap()])

        # ================== LAYER 2 ==================
        p2 = expand_src(
            lambda g, h: pcat_full.ap()[:, g * NSHP + h * GH:
                                        g * NSHP + (h + 1) * GH]
            .partition_broadcast(16), "p")

        pn128 = vdp.tile([128, DR], F32, tag="vf", bufs=2, name="pn128")
        for g in range(NG):
            eng = nc.scalar if g % 2 else nc.sync
            eng.dma_start(pn128[16 * g:16 * (g + 1), :], Pn[:])
        v2dat = vdp.tile([128, DR], F32, tag="vf", bufs=2, name="v2dat")
        nc.vector.tensor_scalar(out=v2dat[:], in0=pn128[:], scalar1=0.0,
                                scalar2=C2c + D2, op0=ALU.max, op1=ALU.mult)
        nc.vector.scalar_tensor_tensor(out=v2dat[:], in0=pn128[:],
                                       scalar=-D2, in1=v2dat[:],
                                       op0=ALU.mult, op1=ALU.add)
        vb2 = expand_dst(v2dat, "v2")

        u2t = mf.tile([128, L2h], F32, tag="mf", bufs=2, name="u2t")
        nc.vector.tensor_scalar(out=u2t[:], in0=p2[:], scalar1=0.0,
                                scalar2=A2 + B2, op0=ALU.max, op1=ALU.mult)
        ep2 = mf.tile([128, L2h], F32, tag="mf", bufs=2, name="ep2")
        nc.vector.scalar_tensor_tensor(out=ep2[:], in0=p2[:], scalar=-B2,
                                       in1=u2t[:], op0=ALU.mult, op1=ALU.add)
        nc.vector.tensor_tensor(out=ep2[:], in0=ep2[:], in1=vb2[:],
                                op=ALU.add)
        nc.vector.scalar_tensor_tensor(out=ep2[:], in0=ep2[:], scalar=0.2,
                                       in1=ep2[:], op0=ALU.mult, op1=ALU.max)
        numer2 = mb.tile([128, L2h], BF16, tag="mb", bufs=6, name="numer2")
        nc.scalar.activation(numer2[:], ep2[:], AF.Exp, bias=g2b[:])
        t0 = mb.tile([128, L2h], BF16, tag="mb", bufs=6, name="t0")
        scan(t0, numer2)
        w2x = mb.tile([128, L2h], BF16, tag="mb", bufs=6, name="w2x")
        nc.vector.tensor_tensor(out=w2x[:], in0=numer2[:], in1=p2[:],
                                op=ALU.mult)
        w21 = mb.tile([128, L2h], BF16, tag="mb", bufs=6, name="w21")
        nc.vector.scalar_tensor_tensor(out=w21[:], in0=p2[:], scalar=0.0,
                                       in1=numer2[:], op0=ALU.max,
                                       op1=ALU.mult)
        t2 = mb.tile([128, L2h], BF16, tag="mb", bufs=6, name="t2")
        scan(t2, w2x)
        t1 = mb.tile([128, L2h], BF16, tag="mb", bufs=6, name="t1")
        scan(t1, w21)

        den2 = nod.tile([16, DR], F32, tag="nd", bufs=6, name="den2")
        Sp = nod.tile([16, DR], F32, tag="nd", bufs=6, name="Sp")
        Sx = nod.tile([16, DR], F32, tag="nd", bufs=6, name="Sx")
        boundary_bf(t0, den2, "d2")
        boundary_bf(t1, Sp, "sp")
        boundary_bf(t2, Sx, "sx")

        rpn = nod.tile([16, DR], F32, tag="nd", bufs=6, name="rpn")
        nc.vector.tensor_scalar(out=rpn[:], in0=Pn[:], scalar1=0.0,
                                scalar2=None, op0=ALU.max)
        e2s = nod.tile([16, DR], F32, tag="nd", bufs=6, name="e2s")
        nc.vector.tensor_scalar(out=e2s[:], in0=rpn[:],
                                scalar1=A2 + B2 + C2c + D2,
                                scalar2=None, op0=ALU.mult)
        nc.vector.scalar_tensor_tensor(out=e2s[:], in0=Pn[:],
                                       scalar=-(B2 + D2), in1=e2s[:],
                                       op0=ALU.mult, op1=ALU.add)
        nc.vector.scalar_tensor_tensor(out=e2s[:], in0=e2s[:], scalar=0.2,
                                       in1=e2s[:], op0=ALU.mult, op1=ALU.max)
        d2s = nod.tile([16, DR], F32, tag="nd", bufs=6, name="d2s")
        nc.scalar.activation(d2s[:], e2s[:], AF.Exp, bias=g2b[0:16, :])
        nc.vector.tensor_tensor(out=den2[:], in0=den2[:], in1=d2s[:],
                                op=ALU.add)
        nc.vector.tensor_tensor(out=e2s[:], in0=d2s[:], in1=rpn[:],
                                op=ALU.mult)
        nc.vector.tensor_tensor(out=Sp[:], in0=Sp[:], in1=e2s[:],
                                op=ALU.add)
        nc.vector.tensor_tensor(out=e2s[:], in0=d2s[:], in1=Pn[:],
                                op=ALU.mult)
        nc.vector.tensor_tensor(out=Sx[:], in0=Sx[:], in1=e2s[:],
                                op=ALU.add)

        nc.vector.tensor_scalar(out=den2[:], in0=den2[:], scalar1=1e-30,
                                scalar2=None, op0=ALU.add)
        rec2 = nod.tile([16, DR], F32, tag="nd", bufs=6, name="rec2")
        nc.vector.reciprocal(rec2[:], den2[:])
        nc.vector.tensor_tensor(out=Sp[:], in0=Sp[:], in1=rec2[:],
                                op=ALU.mult)
        nc.vector.tensor_tensor(out=Sx[:], in0=Sx[:], in1=rec2[:],
                                op=ALU.mult)
        nc.sync.dma_start(rp_d.ap(), Sp[:])
        nc.sync.dma_start(rx_d.ap(), Sx[:])

        rp128 = fy.tile([128, 98], F32, tag="fy", bufs=3, name="rp128")
        nc.sync.dma_start(rp128[:], rp_d.ap())
        rm128 = fy.tile([128, 98], F32, tag="fy", bufs=3, name="rm128")
        nc.sync.dma_start(rm128[:], rx_d.ap())
        nc.vector.tensor_tensor(out=rm128[:], in0=rp128[:], in1=rm128[:],
                                op=ALU.subtract)
        yacc = fy.tile([128, 98], F32, tag="fy", bufs=3, name="yacc")
        nc.vector.memset(yacc[:], float(bl))
        for k in range(20):
            tk = fy.tile([128, 98], F32, tag="yk", bufs=2, name=f"tk{k}")
            nc.vector.tensor_scalar(out=tk[:], in0=rp128[:],
                                    scalar1=float(qp[k]),
                                    scalar2=float(b2[k]), op0=ALU.mult,
                                    op1=ALU.add)
            nc.vector.scalar_tensor_tensor(out=tk[:], in0=rm128[:],
                                           scalar=float(qm[k]), in1=tk[:],
                                           op0=ALU.mult, op1=ALU.add)
            hk = fy.tile([128, 98], F32, tag="yk", bufs=2, name=f"hk{k}")
            nc.scalar.activation(hk[:], tk[:], AF.Relu)
            nc.vector.scalar_tensor_tensor(out=yacc[:], in0=hk[:],
                                           scalar=float(Wl[k]), in1=yacc[:],
                                           op0=ALU.mult, op1=ALU.add)
        nc.sync.dma_start(y_out.ap(), yacc[:])

    nc.compile()
    return nc


def make_in_maps2(pp, inputs):
    x = np.asarray(inputs["x"], np.float32).reshape(-1)
    Nsh = pp["Nsh"]
    xg = np.zeros((NG, NSHP), np.float32)
    for g in range(NG):
        xg[g, :Nsh] = x[g * Nsh:(g + 1) * Nsh]
    xg_hi = np.asarray(xg, BF)
    ind = np.zeros((128, 16), BF)
    for p in range(128):
        ind[p, p % 16] = 1.0
    maps = []
    for c in range(NC):
        xs16 = np.zeros((16, DR), np.float32)
        xs16.reshape(-1)[:Nsh] = x[c * Nsh:(c + 1) * Nsh]
        maps.append({
            "xg_hi": xg_hi, "ind": ind, "xs16": xs16,
            "main_a": np.asarray(pp["main_a"][c], BF),
            "aux_a": np.asarray(pp["aux_a"][c], BF),
            "ends_idx": pp["ends_idx"][c],
            "vplace": pp["vplace"][c].reshape(256, DRH),
            "place": pp["place"][c].reshape(256, GH),
            "perm": pp["perm"][c].reshape(256, 2 * pp["La"]),
        })
    return maps


# ===================== runner =====================

def _run_spmd(nc, maps):
    from concourse.bass_utils import run_bass_kernel_spmd
    return run_bass_kernel_spmd(nc, maps, list(range(8)))


def kernel(**inputs):
    x = np.asarray(inputs["x"], np.float32)
    N = x.shape[0]
    if np.any(np.asarray(inputs["b1"])) or N % NC or (N // NC) > NSHP:
        return _kernel_numpy(**inputs)
    pp = prep2(np.asarray(inputs["edge_index"]), N)
    C = host_consts(inputs, x.reshape(-1))
    nc = build2(pp, C)
    maps = make_in_maps2(pp, inputs)
    res = _run_spmd(nc, maps)
    Nsh = pp["Nsh"]
    y = np.zeros((N, 1), np.float32)
    for c in range(NC):
        y[c * Nsh:(c + 1) * Nsh, 0] = res.results[c]["y"].reshape(-1)[:Nsh]
    return y


def _kernel_numpy(x, edge_index, W1, a_src1, a_dst1, b1, W2, a_src2, a_dst2,
                  b2, Wl, bl):
    def lr(v):
        return np.where(v > 0, v, 0.2 * v).astype(np.float32)

    def conv(h, src, dst, W, asrc, adst, b, n):
        hh = (h @ W).astype(np.float32)
        u, v = hh @ asrc, hh @ adst
        e = lr(u[src] + v[dst])
        m = np.full(n, -np.inf, np.float32)
        np.maximum.at(m, dst, e)
        ee = np.exp(e - m[dst]).astype(np.float32)
        den = np.bincount(dst, weights=ee, minlength=n).astype(np.float32)
        al = ee / (den[dst] + 1e-16)
        out = np.zeros((n, hh.shape[1]), np.float32)
        wh = hh[src] * al[:, None]
        for k in range(hh.shape[1]):
            out[:, k] = np.bincount(dst, weights=wh[:, k], minlength=n)
        return out + b

    n = x.shape[0]
    loop = np.arange(n, dtype=np.int64)
    src = np.concatenate([edge_index[0], loop])
    dst = np.concatenate([edge_index[1], loop])
    h = np.maximum(conv(np.asarray(x, np.float32), src, dst, W1, a_src1,
                        a_dst1, b1, n), 0)
    h = np.maximum(conv(h, src, dst, W2, a_src2, a_dst2, b2, n), 0)
    return (h @ Wl + bl).astype(np.float32)
